# revision 16
# baseline (speedup 1.0000x reference)
"""Trainium2 Bass kernel for 2-layer GAT (nn_GAT_75273596830284).

Strategy (8 NeuronCores, SPMD):
- dst-sharded: core c owns destination nodes [6250c, 6250c+6250). Every edge
  (src, dst) is processed by the core owning dst, so segment softmax and
  aggregation are exact-local (no all-reduce needed).
- Node phase: each core computes h = x_shard @ W1 (+ per-node attention
  scores asrc/adst folded into the same matmul via pre-concatenated weights),
  writes a [6272, 384] bf16 table shard ([h(256)|asrc(8)|adst(8)|pad]), and
  the shards are AllGathered so each core holds the full gather table.
- Edge phase (per 128-dst block): three dma_gathers (src rows from the
  lower/upper table half -- int16 index limit -- and per-edge adst via a
  128-row window gather over the block's own shard rows), one batched one-hot
  build on DVE, batched score/exp, then per 128-edge chunk one bf16 PE
  scatter-accumulate of [msg | p] into a PSUM block accumulator.
- Self-loops are not in the edge stream: their contribution is added
  analytically per block (scores from the node's own asrc+adst, message from
  its own h) via one extra identity matmul into the accumulator.
- Layer 2 repeats the same edge structure with a [6272, 128] bf16 table.

All PE matmuls run in bf16 (4x the fp32 rate); PSUM accumulation stays fp32.
"""
import sys

sys.path.insert(0, "/opt/trn_rl_repo")
import numpy as np
import orjson

N, E0 = 50000, 800000
NFEAT, HID, NHEAD, NCLASS = 256, 32, 8, 40
NCORES = 8
SH = N // NCORES            # 6250 real nodes per core
P = 128
NB = 49                     # blocks per core
SHP = NB * P                # 6272 padded nodes per core
HALF = NCORES * SHP // 2    # 25088: table split point for int16 indices
TROW1 = 384                 # table1 row: h(256)|asrc(8)|adst(8)|pad(112)
TROW2 = 128                 # table2 row: h2(40)|asrc2(1)|adst2(1)|pad(86)
MSHIFT = -16.0              # constant softmax shift (upper bound on scores)


# ---------------------------------------------------------------------------
# BIR post-pass: this container's walrus rejects >1 sync-wait per instruction
# ("Too many sync wait commands"); split excess waits onto NoOp instructions.
_patched = False


def _install_bir_patch():
    global _patched
    if _patched:
        return
    _patched = True
    from concourse import bass as _bass

    orig = _bass.Bass.to_json_bytes

    def _transform(bir, maxw=1):
        for fn in bir.get("functions", []):
            for bb in fn.get("blocks", []):
                out = []
                for ins in bb.get("instructions", []):
                    si = ins.get("sync_info")
                    ws = (si or {}).get("on_wait") or []
                    if len(ws) > maxw:
                        rest, keep = ws[:-maxw], ws[-maxw:]
                        for i in range(0, len(rest), maxw):
                            out.append({
                                "debug": ins.get("debug", 0),
                                "engine": ins["engine"],
                                "ins": [], "outs": [],
                                "name": f"{ins['name']}-ws{i}",
                                "opcode": "NoOp",
                                "sync_info": {"on_update": [],
                                              "on_wait": rest[i:i + maxw]},
                            })
                        si["on_wait"] = keep
                    out.append(ins)
                bb["instructions"] = out

    def patched(self, *a, **kw):
        bir = orjson.loads(orig(self, *a, **kw))
        _transform(bir)
        return orjson.dumps(bir)

    _bass.Bass.to_json_bytes = patched


# ---------------------------------------------------------------------------
_prog_cache = {}


def _build_program(key):
    """Build the SPMD Bass program. key = (C1, C2)."""
    C1, C2 = key
    NCH = C1 + C2
    import concourse.bass as bass
    import concourse.tile as tile
    from concourse import mybir
    from contextlib import ExitStack

    f32 = mybir.dt.float32
    bf16 = mybir.dt.bfloat16
    i16 = mybir.dt.int16
    TOT = NB * NCH

    nc = bass.Bass("TRN2", target_bir_lowering=False, debug=False,
                   num_devices=NCORES, num_swdge_queues=4)

    def din(name, shape, dt=f32):
        return nc.dram_tensor(name, shape, dt, kind="ExternalInput").ap()

    xT = din("xT", [NFEAT, SHP], bf16)
    wcat1 = din("wcat1", [NFEAT, 272], bf16)  # [W1 | W1@Asrc | W1@Adst]
    w2cat = din("w2cat", [NFEAT, 42], bf16)   # [W2 | W2@a_src2 | W2@a_dst2]
    b1rep = din("b1rep", [P, 256])
    b2rep = din("b2rep", [P, NCLASS])
    iota_in = din("iota_row", [P, P], bf16)
    ident_in = din("ident", [P, P], bf16)
    ld_in = din("ld", [P, TOT], bf16)
    slo_in = din("slo", [P, NB * C1 * 8], i16)
    shi_in = din("shi", [P, NB * C2 * 8], i16)

    tshard = nc.dram_tensor("tshard", [SHP, TROW1], bf16).ap()
    tfull = nc.dram_tensor("tfull", [NCORES * SHP, TROW1], bf16).ap()
    t2shard = nc.dram_tensor("t2shard", [SHP, TROW2], bf16).ap()
    t2full = nc.dram_tensor("t2full", [NCORES * SHP, TROW2], bf16).ap()

    fin = nc.dram_tensor("fin", [SHP, NCLASS], f32, kind="ExternalOutput").ap()
    lsm = nc.dram_tensor("lsm", [SHP, NCLASS], f32, kind="ExternalOutput").ap()

    AL = mybir.AluOpType
    AF = mybir.ActivationFunctionType
    groups = [list(range(NCORES))]

    with tile.TileContext(nc, num_cores=NCORES) as tc, ExitStack() as ctx:
        perm = ctx.enter_context(tc.tile_pool(name="perm", bufs=1))
        # --- persistent SBUF state
        iota_sb = perm.tile([P, P], bf16)
        identb_sb = perm.tile([P, P], bf16)
        b1_sb = perm.tile([P, 256], f32)
        b2_sb = perm.tile([P, NCLASS], f32)
        ld_sb = perm.tile([P, TOT], bf16)
        slo_sb = perm.tile([P, NB * C1 * 8], i16)
        shi_sb = perm.tile([P, NB * C2 * 8], i16)
        sa_sb = perm.tile([P, NB * 16], bf16)     # [asrc|adst] per block
        sa2_sb = perm.tile([P, NB * 2], bf16)     # [asrc2|adst2] per block
        h2own_sb = perm.tile([P, NB * NCLASS], bf16)
        h1t0 = perm.tile([P, SHP], bf16)
        h1t1 = perm.tile([P, SHP], bf16)
        w1_sb = perm.tile([P, 2 * 272], bf16)  # k-tiles side by side
        w2_sb = perm.tile([P, 2 * 42], bf16)
        o2_sb = perm.tile([P, NB * NCLASS], f32)
        mmax_sb = perm.tile([P, NB], f32)
        ssum_sb = perm.tile([P, NB], f32)
        msh_sb = perm.tile([P, 1], f32)
        nc.gpsimd.memset(msh_sb[:], MSHIFT)

        nc.sync.dma_start(out=iota_sb[:], in_=iota_in[:])
        nc.sync.dma_start(out=identb_sb[:], in_=ident_in[:])
        nc.sync.dma_start(out=b1_sb[:], in_=b1rep[:])
        nc.sync.dma_start(out=b2_sb[:], in_=b2rep[:])
        nc.sync.dma_start(out=ld_sb[:], in_=ld_in[:])
        nc.sync.dma_start(out=slo_sb[:], in_=slo_in[:])
        nc.sync.dma_start(out=shi_sb[:], in_=shi_in[:])
        nc.sync.dma_start(out=w1_sb[:, 0:272], in_=wcat1[0:P, :])
        nc.sync.dma_start(out=w1_sb[:, 272:544], in_=wcat1[P:2 * P, :])
        nc.sync.dma_start(out=w2_sb[:, 0:42], in_=w2cat[0:P, :])
        nc.sync.dma_start(out=w2_sb[:, 42:84], in_=w2cat[P:2 * P, :])

        # ------------------------------------------------ P1: node phase
        with tc.tile_pool(name="xt", bufs=1) as xtp, \
             tc.tile_pool(name="np1", bufs=3) as np1, \
             tc.tile_pool(name="pp1", bufs=2, space="PSUM") as pp1:
            xt0 = xtp.tile([P, SHP], bf16)
            xt1 = xtp.tile([P, SHP], bf16)
            nc.sync.dma_start(out=xt0[:], in_=xT[0:P, :])
            nc.sync.dma_start(out=xt1[:], in_=xT[P:2 * P, :])
            for nt in range(NB):
                ph = pp1.tile([P, 272], f32, tag="ph")
                nc.tensor.matmul(out=ph[:], lhsT=xt0[:, nt * P:(nt + 1) * P],
                                 rhs=w1_sb[:, 0:272], start=True, stop=False)
                nc.tensor.matmul(out=ph[:], lhsT=xt1[:, nt * P:(nt + 1) * P],
                                 rhs=w1_sb[:, 272:544], start=False, stop=True)
                hx = np1.tile([P, 272], bf16, tag="hx")
                nc.scalar.activation(hx[:], ph[:], AF.Copy)
                nc.sync.dma_start(out=tshard[nt * P:(nt + 1) * P, 0:272],
                                  in_=hx[:])
                nc.vector.tensor_copy(sa_sb[:, nt * 16:(nt + 1) * 16],
                                      hx[:, 256:272])

        # ------------------------------------------------ P2: allgather table1
        nc.gpsimd.collective_compute(
            "AllGather", AL.bypass, replica_groups=groups,
            ins=[tshard[:]], outs=[tfull[:]])

        # shared num_idxs registers for all dma_gathers (to_reg allocates a
        # fresh Pool register per call; 294 calls would exhaust the file)
        rn_lo = nc.gpsimd.to_reg(C1 * P)
        rn_hi = nc.gpsimd.to_reg(C2 * P)

        # ------------------------------------------------ P3: L1 edge phase
        with tc.tile_pool(name="gp", bufs=3) as gp, \
             tc.tile_pool(name="g2p", bufs=3) as g2p, \
             tc.tile_pool(name="ohtp", bufs=4) as ohtp, \
             tc.tile_pool(name="ohp", bufs=3) as ohp, \
             tc.tile_pool(name="mp", bufs=2) as mp, \
             tc.tile_pool(name="sp", bufs=4) as sp, \
             tc.tile_pool(name="fp", bufs=2) as fp, \
             tc.tile_pool(name="pb", bufs=2, space="PSUM") as pbp, \
             tc.tile_pool(name="pt", bufs=2, space="PSUM") as ptp, \
             tc.tile_pool(name="pa", bufs=2, space="PSUM") as pap:
            for b in range(NB):
                g = gp.tile([P, NCH * TROW1], bf16, tag="g")
                nc.gpsimd.dma_gather(
                    g[:, 0:C1 * TROW1].rearrange("p (c w) -> p c w", c=C1),
                    tfull[0:HALF, :],
                    slo_sb[:, b * C1 * 8:(b + 1) * C1 * 8],
                    C1 * P, rn_lo, TROW1, single_packet=False, queue_num=0)
                nc.gpsimd.dma_gather(
                    g[:, C1 * TROW1:].rearrange("p (c w) -> p c w", c=C2),
                    tfull[HALF:NCORES * SHP, :],
                    shi_sb[:, b * C2 * 8:(b + 1) * C2 * 8],
                    C2 * P, rn_hi, TROW1, single_packet=False, queue_num=1)
                g3 = g[:].rearrange("p (c w) -> p c w", c=NCH)
                # batched one-hot build: oh[e, ch, slot]
                oh = ohp.tile([P, NCH * P], bf16, tag="oh")
                nc.vector.tensor_tensor(
                    out=oh[:].rearrange("p (c s) -> p c s", c=NCH),
                    in0=ld_sb[:, b * NCH:(b + 1) * NCH].unsqueeze(-1)
                        .to_broadcast([P, NCH, P]),
                    in1=iota_sb[:].unsqueeze(1).to_broadcast([P, NCH, P]),
                    op=AL.is_equal)
                # per-edge adst via PE: transpose one-hot, matmul vs adst_blk
                psc = pap.tile([P, NCH * 8], f32, tag="psc")
                for ch in range(NCH):
                    pst = ptp.tile([P, P], bf16, tag="pst")
                    nc.tensor.transpose(out=pst[:],
                                        in_=oh[:, ch * P:(ch + 1) * P],
                                        identity=identb_sb[:])
                    oht = ohtp.tile([P, P], bf16, tag="oht")
                    nc.scalar.activation(oht[:], pst[:], AF.Copy)
                    nc.tensor.matmul(out=psc[:, ch * 8:(ch + 1) * 8],
                                     lhsT=oht[:],
                                     rhs=sa_sb[:, b * 16 + 8:b * 16 + 16],
                                     start=True, stop=True)
                # batched scores: esc = asrc[src] + adst[dst]; lrelu; exp
                esc = sp.tile([P, NCH * 8], f32, tag="esc")
                nc.vector.tensor_tensor(
                    out=esc[:].rearrange("p (c h) -> p c h", c=NCH),
                    in0=psc[:].rearrange("p (c h) -> p c h", c=NCH),
                    in1=g3[:, :, 256:264], op=AL.add)
                es2 = sp.tile([P, NCH * 8], f32, tag="es2")
                nc.vector.tensor_scalar_mul(es2[:], esc[:], 0.2)
                es3 = sp.tile([P, NCH * 8], f32, tag="es3")
                nc.vector.tensor_tensor(out=es3[:], in0=esc[:], in1=es2[:],
                                        op=AL.max)
                msg = mp.tile([P, NCH * 264], bf16, tag="msg")
                m3 = msg[:].rearrange("p (c w) -> p c w", c=NCH)
                nc.scalar.activation(m3[:, :, 256:264],
                                     es3[:].rearrange("p (c h) -> p c h",
                                                      c=NCH),
                                     AF.Exp, bias=msh_sb[:])
                nc.vector.tensor_tensor(
                    out=m3[:, :, 0:256].rearrange("p c (h w) -> p c h w", h=8),
                    in0=g3[:, :, 0:256].rearrange("p c (h w) -> p c h w", h=8),
                    in1=m3[:, :, 256:264].unsqueeze(-1)
                        .to_broadcast([P, NCH, 8, 32]),
                    op=AL.mult)
                # self-loop contribution for this block
                hself = fp.tile([P, 256], bf16, tag="hself")
                nc.sync.dma_start(out=hself[:],
                                  in_=tshard[b * P:(b + 1) * P, 0:256])
                sl = sp.tile([P, 8], f32, tag="sl")
                nc.vector.tensor_tensor(out=sl[:],
                                        in0=sa_sb[:, b * 16:b * 16 + 8],
                                        in1=sa_sb[:, b * 16 + 8:b * 16 + 16],
                                        op=AL.add)
                sl2 = sp.tile([P, 8], f32, tag="sl2")
                nc.vector.tensor_scalar_mul(sl2[:], sl[:], 0.2)
                sl3 = sp.tile([P, 8], f32, tag="sl3")
                nc.vector.tensor_tensor(out=sl3[:], in0=sl[:], in1=sl2[:],
                                        op=AL.max)
                slt = mp.tile([P, 264], bf16, tag="slt")
                nc.scalar.activation(slt[:, 256:264], sl3[:], AF.Exp,
                                     bias=msh_sb[:])
                nc.vector.tensor_tensor(
                    out=slt[:, 0:256].rearrange("p (h w) -> p h w", h=8),
                    in0=hself[:].rearrange("p (h w) -> p h w", h=8),
                    in1=slt[:, 256:264].unsqueeze(-1)
                        .to_broadcast([P, 8, 32]),
                    op=AL.mult)
                # scatter-accumulate: self-loop first, then edge chunks
                pblk = pbp.tile([P, 264], f32, tag="pblk")
                nc.tensor.matmul(out=pblk[:], lhsT=identb_sb[:], rhs=slt[:],
                                 start=True, stop=False)
                for ch in range(NCH):
                    nc.tensor.matmul(out=pblk[:],
                                     lhsT=oh[:, ch * P:(ch + 1) * P],
                                     rhs=msg[:, ch * 264:(ch + 1) * 264],
                                     start=False, stop=(ch == NCH - 1))
                # block finalize: normalize, bias, elu
                den = sp.tile([P, 8], f32, tag="den")
                nc.vector.tensor_scalar_add(den[:], pblk[:, 256:264], 1e-16)
                rcp = sp.tile([P, 8], f32, tag="rcp")
                nc.vector.reciprocal(rcp[:], den[:])
                xb = fp.tile([P, 256], f32, tag="xb")
                nc.vector.tensor_tensor(
                    out=xb[:].rearrange("p (h c) -> p h c", h=8),
                    in0=pblk[:, 0:256].rearrange("p (h c) -> p h c", h=8),
                    in1=rcp[:].unsqueeze(-1).to_broadcast([P, 8, 32]),
                    op=AL.mult)
                nc.vector.tensor_tensor(out=xb[:], in0=xb[:], in1=b1_sb[:],
                                        op=AL.add)
                m0 = fp.tile([P, 256], f32, tag="m0")
                nc.vector.tensor_scalar(out=m0[:], in0=xb[:], scalar1=0.0,
                                        scalar2=None, op0=AL.min)
                ex = fp.tile([P, 256], f32, tag="ex")
                nc.scalar.activation(ex[:], m0[:], AF.Exp)
                nc.vector.tensor_scalar_add(ex[:], ex[:], -1.0)
                nc.vector.tensor_scalar(out=m0[:], in0=xb[:], scalar1=0.0,
                                        scalar2=None, op0=AL.max)
                h1b = fp.tile([P, 256], bf16, tag="h1b")
                nc.vector.tensor_tensor(out=h1b[:], in0=m0[:], in1=ex[:],
                                        op=AL.add)
                # transposes for layer-2 matmul
                for half, dstt in ((0, h1t0), (1, h1t1)):
                    pst2 = ptp.tile([P, P], bf16, tag="pst")
                    nc.tensor.transpose(
                        out=pst2[:],
                        in_=h1b[:, half * P:(half + 1) * P],
                        identity=identb_sb[:])
                    nc.scalar.activation(dstt[:, b * P:(b + 1) * P], pst2[:],
                                         AF.Copy)

            # -------------------------------------------- P4: layer-2 nodes
            for b in range(NB):
                p2 = pap.tile([P, 42], f32, tag="psc")
                nc.tensor.matmul(out=p2[:], lhsT=h1t0[:, b * P:(b + 1) * P],
                                 rhs=w2_sb[:, 0:42], start=True, stop=False)
                nc.tensor.matmul(out=p2[:], lhsT=h1t1[:, b * P:(b + 1) * P],
                                 rhs=w2_sb[:, 42:84], start=False, stop=True)
                h2x = sp.tile([P, 42], bf16, tag="h2x")
                nc.scalar.activation(h2x[:], p2[:], AF.Copy)
                nc.sync.dma_start(out=t2shard[b * P:(b + 1) * P, 0:42],
                                  in_=h2x[:])
                nc.vector.tensor_copy(sa2_sb[:, b * 2:(b + 1) * 2],
                                      h2x[:, 40:42])
                nc.vector.tensor_copy(h2own_sb[:, b * 40:(b + 1) * 40],
                                      h2x[:, 0:40])

            # -------------------------------------------- P5: allgather table2
            nc.gpsimd.collective_compute(
                "AllGather", AL.bypass, replica_groups=groups,
                ins=[t2shard[:]], outs=[t2full[:]])

            # -------------------------------------------- P6: L2 edge phase
            for b in range(NB):
                g2 = g2p.tile([P, NCH * TROW2], bf16, tag="g2")
                nc.gpsimd.dma_gather(
                    g2[:, 0:C1 * TROW2].rearrange("p (c w) -> p c w", c=C1),
                    t2full[0:HALF, :],
                    slo_sb[:, b * C1 * 8:(b + 1) * C1 * 8],
                    C1 * P, rn_lo, TROW2, single_packet=False, queue_num=0)
                nc.gpsimd.dma_gather(
                    g2[:, C1 * TROW2:].rearrange("p (c w) -> p c w", c=C2),
                    t2full[HALF:NCORES * SHP, :],
                    shi_sb[:, b * C2 * 8:(b + 1) * C2 * 8],
                    C2 * P, rn_hi, TROW2, single_packet=False, queue_num=1)
                g23 = g2[:].rearrange("p (c w) -> p c w", c=NCH)
                oh = ohp.tile([P, NCH * P], bf16, tag="oh")
                nc.vector.tensor_tensor(
                    out=oh[:].rearrange("p (c s) -> p c s", c=NCH),
                    in0=ld_sb[:, b * NCH:(b + 1) * NCH].unsqueeze(-1)
                        .to_broadcast([P, NCH, P]),
                    in1=iota_sb[:].unsqueeze(1).to_broadcast([P, NCH, P]),
                    op=AL.is_equal)
                psc2 = pap.tile([P, NCH * 8], f32, tag="psc")
                for ch in range(NCH):
                    pst = ptp.tile([P, P], bf16, tag="pst")
                    nc.tensor.transpose(out=pst[:],
                                        in_=oh[:, ch * P:(ch + 1) * P],
                                        identity=identb_sb[:])
                    oht = ohtp.tile([P, P], bf16, tag="oht")
                    nc.scalar.activation(oht[:], pst[:], AF.Copy)
                    nc.tensor.matmul(out=psc2[:, ch:ch + 1], lhsT=oht[:],
                                     rhs=sa2_sb[:, b * 2 + 1:b * 2 + 2],
                                     start=True, stop=True)
                esc = sp.tile([P, NCH], f32, tag="esc1")
                nc.vector.tensor_tensor(
                    out=esc[:].unsqueeze(-1),
                    in0=psc2[:, 0:NCH].unsqueeze(-1),
                    in1=g23[:, :, 40:41], op=AL.add)
                es2 = sp.tile([P, NCH], f32, tag="es21")
                nc.vector.tensor_scalar_mul(es2[:], esc[:], 0.2)
                es3 = sp.tile([P, NCH], f32, tag="es31")
                nc.vector.tensor_tensor(out=es3[:], in0=esc[:], in1=es2[:],
                                        op=AL.max)
                msg2 = mp.tile([P, NCH * 41], bf16, tag="msg2")
                m23 = msg2[:].rearrange("p (c w) -> p c w", c=NCH)
                nc.scalar.activation(m23[:, :, 40:41],
                                     es3[:].unsqueeze(-1),
                                     AF.Exp, bias=msh_sb[:])
                nc.vector.tensor_tensor(
                    out=m23[:, :, 0:40], in0=g23[:, :, 0:40],
                    in1=m23[:, :, 40:41].to_broadcast([P, NCH, 40]),
                    op=AL.mult)
                # self-loop contribution
                sl = sp.tile([P, 1], f32, tag="sl1")
                nc.vector.tensor_tensor(out=sl[:],
                                        in0=sa2_sb[:, b * 2:b * 2 + 1],
                                        in1=sa2_sb[:, b * 2 + 1:b * 2 + 2],
                                        op=AL.add)
                sl2 = sp.tile([P, 1], f32, tag="sl21")
                nc.vector.tensor_scalar_mul(sl2[:], sl[:], 0.2)
                sl3 = sp.tile([P, 1], f32, tag="sl31")
                nc.vector.tensor_tensor(out=sl3[:], in0=sl[:], in1=sl2[:],
                                        op=AL.max)
                slt2 = mp.tile([P, 41], bf16, tag="slt2")
                nc.scalar.activation(slt2[:, 40:41], sl3[:], AF.Exp,
                                     bias=msh_sb[:])
                nc.vector.tensor_tensor(
                    out=slt2[:, 0:40],
                    in0=h2own_sb[:, b * 40:(b + 1) * 40],
                    in1=slt2[:, 40:41].to_broadcast([P, 40]),
                    op=AL.mult)
                pblk2 = pbp.tile([P, 41], f32, tag="pblk")
                nc.tensor.matmul(out=pblk2[:], lhsT=identb_sb[:],
                                 rhs=slt2[:], start=True, stop=False)
                for ch in range(NCH):
                    nc.tensor.matmul(out=pblk2[:],
                                     lhsT=oh[:, ch * P:(ch + 1) * P],
                                     rhs=msg2[:, ch * 41:(ch + 1) * 41],
                                     start=False, stop=(ch == NCH - 1))
                den = sp.tile([P, 1], f32, tag="den1")
                nc.vector.tensor_scalar_add(den[:], pblk2[:, 40:41], 1e-16)
                rcp = sp.tile([P, 1], f32, tag="rcp1")
                nc.vector.reciprocal(rcp[:], den[:])
                o2 = o2_sb[:, b * NCLASS:(b + 1) * NCLASS]
                nc.vector.tensor_tensor(out=o2, in0=pblk2[:, 0:40],
                                        in1=rcp[:].to_broadcast([P, 40]),
                                        op=AL.mult)
                nc.vector.tensor_tensor(out=o2, in0=o2, in1=b2_sb[:], op=AL.add)
                nc.vector.tensor_reduce(out=mmax_sb[:, b:b + 1], in_=o2,
                                        axis=mybir.AxisListType.X, op=AL.max)
                negm = sp.tile([P, 1], f32, tag="negm")
                nc.vector.tensor_scalar_mul(negm[:], mmax_sb[:, b:b + 1], -1.0)
                et = sp.tile([P, NCLASS], f32, tag="et")
                nc.scalar.activation(et[:], o2, AF.Exp, bias=negm[:],
                                     accum_out=ssum_sb[:, b:b + 1])

            # batched log + final outputs
            lnS = perm.tile([P, NB], f32)
            nc.scalar.activation(lnS[:], ssum_sb[:], AF.Ln)
            q = perm.tile([P, NB], f32)
            nc.vector.tensor_tensor(out=q[:], in0=mmax_sb[:], in1=lnS[:],
                                    op=AL.add)
            for b in range(NB):
                o2 = o2_sb[:, b * NCLASS:(b + 1) * NCLASS]
                lsb = sp.tile([P, NCLASS], f32, tag="lsb")
                nc.vector.tensor_tensor(out=lsb[:], in0=o2,
                                        in1=q[:, b:b + 1].to_broadcast([P, 40]),
                                        op=AL.subtract)
                nc.sync.dma_start(out=fin[b * P:(b + 1) * P, :], in_=o2)
                nc.sync.dma_start(out=lsm[b * P:(b + 1) * P, :], in_=lsb[:])

    # dma_gather needs the 'mlp' GPSIMD ucode library; raw Bass doesn't run
    # Bacc's insert_library_loads pass, so run it (plus ISA codegen) here.
    from concourse.library_config import all_libraries, standard
    import bass_rust as _br
    mask = {}
    for lib_ in all_libraries:
        for it in lib_.instructions:
            mask[it] = mask.get(it, 0) | (1 << lib_.index)
    _br.insert_library_loads(nc, mask, len(all_libraries), standard.index)
    mybir.codegen_inst_isa_subclasses(nc)
    return nc


# ---------------------------------------------------------------------------
def _host_prep(x, edge_index, W1, att_src1, att_dst1, b1, W2, att_src2,
               att_dst2, b2):
    x = np.asarray(x, dtype=np.float32)
    ei = np.asarray(edge_index)
    W1 = np.asarray(W1, np.float32)
    W2 = np.asarray(W2, np.float32)
    a_s1 = np.asarray(att_src1, np.float32)
    a_d1 = np.asarray(att_dst1, np.float32)
    a_s2 = np.asarray(att_src2, np.float32).reshape(-1)
    a_d2 = np.asarray(att_dst2, np.float32).reshape(-1)
    b1 = np.asarray(b1, np.float32)
    b2 = np.asarray(b2, np.float32)

    import ml_dtypes
    bf = ml_dtypes.bfloat16

    # self-loops are handled analytically on-device; edge stream = real edges
    src = np.asarray(ei[0], np.int64)
    dst = np.asarray(ei[1], np.int64)

    core = dst // SH
    local = dst - core * SH
    block = local // P
    slot = (local % P).astype(np.int64)
    cb = core * NB + block
    remap = ((src // SH) * SHP + (src % SH)).astype(np.int64)
    hi = (remap >= HALF).astype(np.int64)

    grp = cb * 2 + hi
    order = np.argsort(grp, kind="stable")
    grp_s = grp[order]
    remap_s = remap[order]
    slot_s = slot[order]
    cb_s = cb[order]
    hi_s = hi[order]

    gcounts = np.bincount(grp_s, minlength=NCORES * NB * 2)
    goffs = np.zeros(NCORES * NB * 2 + 1, np.int64)
    goffs[1:] = np.cumsum(gcounts)
    pos = np.arange(order.size, dtype=np.int64) - goffs[grp_s]

    cntlo = gcounts[0::2].reshape(NCORES, NB)
    cnthi = gcounts[1::2].reshape(NCORES, NB)
    C1 = int(np.ceil(cntlo.max() / P))
    C2 = int(np.ceil(cnthi.max() / P))
    NCH = C1 + C2
    TOT = NB * NCH

    corev = cb_s // NB
    blockv = cb_s % NB
    k = pos + hi_s * (C1 * P)          # global slot index within block
    lane = k % P
    chunk = k // P

    ld_all = np.full((NCORES, P, TOT), -1.0, np.float32)
    ld_all[corev, lane, blockv * NCH + chunk] = slot_s.astype(np.float32)

    # int16 index arrays, wrapped 16 ways: flat[j] -> arr[j%16, j//16]
    slo = np.zeros((NCORES, P, NB * C1 * 8), np.int16)
    shi_a = np.zeros((NCORES, P, NB * C2 * 8), np.int16)

    # indices live in partitions 0-15 (RX Q7 core) AND 16-31 (TX Q7 core)
    lo_m = hi_s == 0
    hi_m = hi_s == 1
    for poff in range(0, P, 16):
        slo[corev[lo_m], poff + pos[lo_m] % 16,
            blockv[lo_m] * (C1 * 8) + pos[lo_m] // 16] = remap_s[lo_m]
        shi_a[corev[hi_m], poff + pos[hi_m] % 16,
              blockv[hi_m] * (C2 * 8) + pos[hi_m] // 16] = \
            (remap_s[hi_m] - HALF)

    # weights
    wa_s1 = np.zeros((NFEAT, 8), np.float32)
    wa_d1 = np.zeros((NFEAT, 8), np.float32)
    for h in range(NHEAD):
        wa_s1[:, h] = W1[:, h * HID:(h + 1) * HID] @ a_s1[h]
        wa_d1[:, h] = W1[:, h * HID:(h + 1) * HID] @ a_d1[h]
    wcat1 = np.concatenate([W1, wa_s1, wa_d1], axis=1)          # [256, 272]
    w2cat = np.concatenate([W2, (W2 @ a_s2)[:, None],
                            (W2 @ a_d2)[:, None]], axis=1)      # [256, 42]

    iota_row = np.broadcast_to(np.arange(P, dtype=np.float32),
                               (P, P)).astype(bf)
    ident = np.eye(P, dtype=bf)
    b1rep = np.broadcast_to(b1, (P, 256)).copy()
    b2rep = np.broadcast_to(b2, (P, NCLASS)).copy()

    in_maps = []
    for c in range(NCORES):
        xs = np.zeros((SHP, NFEAT), np.float32)
        xs[:SH] = x[c * SH:(c + 1) * SH]
        in_maps.append({
            "xT": np.ascontiguousarray(xs.T).astype(bf),
            "wcat1": wcat1.astype(bf), "w2cat": w2cat.astype(bf),
            "b1rep": b1rep, "b2rep": b2rep,
            "iota_row": iota_row, "ident": ident,
            "ld": np.ascontiguousarray(ld_all[c]).astype(bf),
            "slo": np.ascontiguousarray(slo[c]),
            "shi": np.ascontiguousarray(shi_a[c]),
        })
    return (C1, C2), in_maps


def kernel(**inputs):
    _install_bir_patch()
    from concourse.bass_utils import run_bass_kernel_spmd

    key, in_maps = _host_prep(
        inputs["x"], inputs["edge_index"], inputs["W1"], inputs["att_src1"],
        inputs["att_dst1"], inputs["b1"], inputs["W2"], inputs["att_src2"],
        inputs["att_dst2"], inputs["b2"])

    if key not in _prog_cache:
        _prog_cache[key] = _build_program(key)
    nc = _prog_cache[key]

    res = run_bass_kernel_spmd(nc, in_maps, list(range(NCORES)))
    fin = np.concatenate([res.results[c]["fin"][:SH] for c in range(NCORES)])
    lsm = np.concatenate([res.results[c]["lsm"][:SH] for c in range(NCORES)])
    return fin, lsm


# revision 17
# speedup vs baseline: 1.2910x; 1.2910x over previous
"""Trainium2 Bass kernel for 2-layer GAT (nn_GAT_75273596830284).

Strategy (8 NeuronCores, SPMD):
- dst-sharded: core c owns destination nodes [6250c, 6250c+6250). Every edge
  (src, dst) is processed by the core owning dst, so segment softmax and
  aggregation are exact-local (no all-reduce needed).
- Node phase: each core computes h = x_shard @ W1 (+ per-node attention
  scores asrc/adst folded into the same matmul via pre-concatenated weights),
  writes a [6272, 384] bf16 table shard ([h(256)|asrc(8)|adst(8)|pad]), and
  the shards are AllGathered so each core holds the full gather table.
- Edge phase (per 128-dst block): three dma_gathers (src rows from the
  lower/upper table half -- int16 index limit -- and per-edge adst via a
  128-row window gather over the block's own shard rows), one batched one-hot
  build on DVE, batched score/exp, then per 128-edge chunk one bf16 PE
  scatter-accumulate of [msg | p] into a PSUM block accumulator.
- Self-loops are not in the edge stream: their contribution is added
  analytically per block (scores from the node's own asrc+adst, message from
  its own h) via one extra identity matmul into the accumulator.
- Layer 2 repeats the same edge structure with a [6272, 128] bf16 table.

All PE matmuls run in bf16 (4x the fp32 rate); PSUM accumulation stays fp32.
"""
import sys

sys.path.insert(0, "/opt/trn_rl_repo")
import numpy as np
import orjson

N, E0 = 50000, 800000
NFEAT, HID, NHEAD, NCLASS = 256, 32, 8, 40
NCORES = 8
SH = N // NCORES            # 6250 real nodes per core
P = 128
NB = 49                     # blocks per core
SHP = NB * P                # 6272 padded nodes per core
HALF = NCORES * SHP // 2    # 25088: table split point for int16 indices
TROW1 = 384                 # table1 row: h(256)|asrc(8)|adst(8)|pad(112)
TROW2 = 128                 # table2 row: h2(40)|asrc2(1)|adst2(1)|pad(86)
MSHIFT = -16.0              # constant softmax shift (upper bound on scores)


# ---------------------------------------------------------------------------
# BIR post-pass: this container's walrus rejects >1 sync-wait per instruction
# ("Too many sync wait commands"); split excess waits onto NoOp instructions.
_patched = False


def _install_bir_patch():
    global _patched
    if _patched:
        return
    _patched = True
    from concourse import bass as _bass

    orig = _bass.Bass.to_json_bytes

    def _transform(bir, maxw=1):
        for fn in bir.get("functions", []):
            for bb in fn.get("blocks", []):
                out = []
                for ins in bb.get("instructions", []):
                    si = ins.get("sync_info")
                    ws = (si or {}).get("on_wait") or []
                    if len(ws) > maxw:
                        rest, keep = ws[:-maxw], ws[-maxw:]
                        for i in range(0, len(rest), maxw):
                            out.append({
                                "debug": ins.get("debug", 0),
                                "engine": ins["engine"],
                                "ins": [], "outs": [],
                                "name": f"{ins['name']}-ws{i}",
                                "opcode": "NoOp",
                                "sync_info": {"on_update": [],
                                              "on_wait": rest[i:i + maxw]},
                            })
                        si["on_wait"] = keep
                    out.append(ins)
                bb["instructions"] = out

    def patched(self, *a, **kw):
        bir = orjson.loads(orig(self, *a, **kw))
        _transform(bir)
        return orjson.dumps(bir)

    _bass.Bass.to_json_bytes = patched


# ---------------------------------------------------------------------------
_prog_cache = {}


def _build_program(key):
    """Build the SPMD Bass program. key = (C1, C2)."""
    C1, C2 = key
    NCH = C1 + C2
    import concourse.bass as bass
    import concourse.tile as tile
    from concourse import mybir
    from contextlib import ExitStack

    f32 = mybir.dt.float32
    bf16 = mybir.dt.bfloat16
    i16 = mybir.dt.int16
    TOT = NB * NCH

    nc = bass.Bass("TRN2", target_bir_lowering=False, debug=False,
                   num_devices=NCORES, num_swdge_queues=4)

    def din(name, shape, dt=f32):
        return nc.dram_tensor(name, shape, dt, kind="ExternalInput").ap()

    xT = din("xT", [NFEAT, SHP], bf16)
    wcat1 = din("wcat1", [NFEAT, 272], bf16)  # [W1 | W1@Asrc | W1@Adst]
    w2cat = din("w2cat", [NFEAT, 42], bf16)   # [W2 | W2@a_src2 | W2@a_dst2]
    b1rep = din("b1rep", [P, 256])
    b2rep = din("b2rep", [P, NCLASS])
    iota_in = din("iota_row", [P, P], bf16)
    ident_in = din("ident", [P, P], bf16)
    ld_in = din("ld", [P, TOT], bf16)
    slo_in = din("slo", [P, NB * C1 * 8], i16)
    shi_in = din("shi", [P, NB * C2 * 8], i16)
    sw_in = din("sw", [P, NB * NCH * 8], i16)

    tshard = nc.dram_tensor("tshard", [SHP, TROW1], bf16).ap()
    tfull = nc.dram_tensor("tfull", [NCORES * SHP, TROW1], bf16).ap()
    t2shard = nc.dram_tensor("t2shard", [SHP, TROW2], bf16).ap()
    t2full = nc.dram_tensor("t2full", [NCORES * SHP, TROW2], bf16).ap()

    fin = nc.dram_tensor("fin", [SHP, NCLASS], f32, kind="ExternalOutput").ap()
    lsm = nc.dram_tensor("lsm", [SHP, NCLASS], f32, kind="ExternalOutput").ap()

    AL = mybir.AluOpType
    AF = mybir.ActivationFunctionType
    groups = [list(range(NCORES))]

    with tile.TileContext(nc, num_cores=NCORES) as tc, ExitStack() as ctx:
        perm = ctx.enter_context(tc.tile_pool(name="perm", bufs=1))
        # --- persistent SBUF state
        iota_sb = perm.tile([P, P], bf16)
        identb_sb = perm.tile([P, P], bf16)
        b1_sb = perm.tile([P, 256], f32)
        b2_sb = perm.tile([P, NCLASS], f32)
        ld_sb = perm.tile([P, TOT], bf16)
        slo_sb = perm.tile([P, NB * C1 * 8], i16)
        shi_sb = perm.tile([P, NB * C2 * 8], i16)
        sw_sb = perm.tile([P, NB * NCH * 8], i16)
        sa_sb = perm.tile([P, NB * 16], bf16)     # [asrc|adst] per block
        sa2_sb = perm.tile([P, NB * 2], bf16)     # [asrc2|adst2] per block
        h2own_sb = perm.tile([P, NB * NCLASS], bf16)
        h1t0 = perm.tile([P, SHP], bf16)
        h1t1 = perm.tile([P, SHP], bf16)
        w1_sb = perm.tile([P, 2 * 272], bf16)  # k-tiles side by side
        w2_sb = perm.tile([P, 2 * 42], bf16)
        o2_sb = perm.tile([P, NB * NCLASS], f32)
        mmax_sb = perm.tile([P, NB], f32)
        ssum_sb = perm.tile([P, NB], f32)
        msh_sb = perm.tile([P, 1], f32)
        nc.gpsimd.memset(msh_sb[:], MSHIFT)

        nc.sync.dma_start(out=iota_sb[:], in_=iota_in[:])
        nc.sync.dma_start(out=identb_sb[:], in_=ident_in[:])
        nc.sync.dma_start(out=b1_sb[:], in_=b1rep[:])
        nc.sync.dma_start(out=b2_sb[:], in_=b2rep[:])
        nc.sync.dma_start(out=ld_sb[:], in_=ld_in[:])
        nc.sync.dma_start(out=slo_sb[:], in_=slo_in[:])
        nc.sync.dma_start(out=shi_sb[:], in_=shi_in[:])
        nc.sync.dma_start(out=sw_sb[:], in_=sw_in[:])
        nc.sync.dma_start(out=w1_sb[:, 0:272], in_=wcat1[0:P, :])
        nc.sync.dma_start(out=w1_sb[:, 272:544], in_=wcat1[P:2 * P, :])
        nc.sync.dma_start(out=w2_sb[:, 0:42], in_=w2cat[0:P, :])
        nc.sync.dma_start(out=w2_sb[:, 42:84], in_=w2cat[P:2 * P, :])

        # ------------------------------------------------ P1: node phase
        with tc.tile_pool(name="xt", bufs=1) as xtp, \
             tc.tile_pool(name="np1", bufs=3) as np1, \
             tc.tile_pool(name="pp1", bufs=2, space="PSUM") as pp1:
            xt0 = xtp.tile([P, SHP], bf16)
            xt1 = xtp.tile([P, SHP], bf16)
            nc.sync.dma_start(out=xt0[:], in_=xT[0:P, :])
            nc.sync.dma_start(out=xt1[:], in_=xT[P:2 * P, :])
            for nt in range(NB):
                ph = pp1.tile([P, 272], f32, tag="ph")
                nc.tensor.matmul(out=ph[:], lhsT=xt0[:, nt * P:(nt + 1) * P],
                                 rhs=w1_sb[:, 0:272], start=True, stop=False)
                nc.tensor.matmul(out=ph[:], lhsT=xt1[:, nt * P:(nt + 1) * P],
                                 rhs=w1_sb[:, 272:544], start=False, stop=True)
                hx = np1.tile([P, 272], bf16, tag="hx")
                nc.scalar.activation(hx[:], ph[:], AF.Copy)
                nc.sync.dma_start(out=tshard[nt * P:(nt + 1) * P, 0:272],
                                  in_=hx[:])
                nc.vector.tensor_copy(sa_sb[:, nt * 16:(nt + 1) * 16],
                                      hx[:, 256:272])

        # ------------------------------------------------ P2: allgather table1
        nc.gpsimd.collective_compute(
            "AllGather", AL.bypass, replica_groups=groups,
            ins=[tshard[:]], outs=[tfull[:]])

        # shared num_idxs registers for all dma_gathers (to_reg allocates a
        # fresh Pool register per call; 294 calls would exhaust the file)
        rn_lo = nc.gpsimd.to_reg(C1 * P)
        rn_hi = nc.gpsimd.to_reg(C2 * P)
        rn_w = nc.gpsimd.to_reg(NCH * P)

        # ------------------------------------------------ P3: L1 edge phase
        with tc.tile_pool(name="gp", bufs=3) as gp, \
             tc.tile_pool(name="g2p", bufs=3) as g2p, \
             tc.tile_pool(name="gap", bufs=4) as gap, \
             tc.tile_pool(name="ohp", bufs=3) as ohp, \
             tc.tile_pool(name="mp", bufs=2) as mp, \
             tc.tile_pool(name="sp", bufs=4) as sp, \
             tc.tile_pool(name="fp", bufs=2) as fp, \
             tc.tile_pool(name="pb", bufs=2, space="PSUM") as pbp, \
             tc.tile_pool(name="pt", bufs=2, space="PSUM") as ptp, \
             tc.tile_pool(name="pa", bufs=2, space="PSUM") as pap:
            for b in range(NB):
                g = gp.tile([P, NCH * TROW1], bf16, tag="g")
                nc.gpsimd.dma_gather(
                    g[:, 0:C1 * TROW1].rearrange("p (c w) -> p c w", c=C1),
                    tfull[0:HALF, :],
                    slo_sb[:, b * C1 * 8:(b + 1) * C1 * 8],
                    C1 * P, rn_lo, TROW1, single_packet=False, queue_num=0)
                nc.gpsimd.dma_gather(
                    g[:, C1 * TROW1:].rearrange("p (c w) -> p c w", c=C2),
                    tfull[HALF:NCORES * SHP, :],
                    shi_sb[:, b * C2 * 8:(b + 1) * C2 * 8],
                    C2 * P, rn_hi, TROW1, single_packet=False, queue_num=1)
                ga = gap.tile([P, NCH * P], bf16, tag="ga")
                nc.gpsimd.dma_gather(
                    ga[:, 0:C1 * P].rearrange("p (c w) -> p c w", c=C1),
                    tshard[b * P:(b + 1) * P, 256:TROW1],
                    sw_sb[:, b * NCH * 8:b * NCH * 8 + C1 * 8],
                    C1 * P, rn_lo, P, elem_step=TROW1, single_packet=False,
                    queue_num=2)
                nc.gpsimd.dma_gather(
                    ga[:, C1 * P:].rearrange("p (c w) -> p c w", c=C2),
                    tshard[b * P:(b + 1) * P, 256:TROW1],
                    sw_sb[:, b * NCH * 8 + C1 * 8:(b + 1) * NCH * 8],
                    C2 * P, rn_hi, P, elem_step=TROW1, single_packet=False,
                    queue_num=3)
                g3 = g[:].rearrange("p (c w) -> p c w", c=NCH)
                ga3 = ga[:].rearrange("p (c w) -> p c w", c=NCH)
                # batched one-hot build: oh[e, ch, slot]
                oh = ohp.tile([P, NCH * P], bf16, tag="oh")
                nc.vector.tensor_tensor(
                    out=oh[:].rearrange("p (c s) -> p c s", c=NCH),
                    in0=ld_sb[:, b * NCH:(b + 1) * NCH].unsqueeze(-1)
                        .to_broadcast([P, NCH, P]),
                    in1=iota_sb[:].unsqueeze(1).to_broadcast([P, NCH, P]),
                    op=AL.is_equal)
                # batched scores: esc = asrc[src] + adst[dst]; lrelu; exp
                esc = sp.tile([P, NCH * 8], f32, tag="esc")
                nc.vector.tensor_tensor(
                    out=esc[:].rearrange("p (c h) -> p c h", c=NCH),
                    in0=g3[:, :, 256:264], in1=ga3[:, :, 8:16], op=AL.add)
                es2 = sp.tile([P, NCH * 8], f32, tag="es2")
                nc.vector.tensor_scalar_mul(es2[:], esc[:], 0.2)
                es3 = sp.tile([P, NCH * 8], f32, tag="es3")
                nc.vector.tensor_tensor(out=es3[:], in0=esc[:], in1=es2[:],
                                        op=AL.max)
                msg = mp.tile([P, NCH * 264], bf16, tag="msg")
                m3 = msg[:].rearrange("p (c w) -> p c w", c=NCH)
                nc.scalar.activation(m3[:, :, 256:264],
                                     es3[:].rearrange("p (c h) -> p c h",
                                                      c=NCH),
                                     AF.Exp, bias=msh_sb[:])
                nc.vector.tensor_tensor(
                    out=m3[:, :, 0:256].rearrange("p c (h w) -> p c h w", h=8),
                    in0=g3[:, :, 0:256].rearrange("p c (h w) -> p c h w", h=8),
                    in1=m3[:, :, 256:264].unsqueeze(-1)
                        .to_broadcast([P, NCH, 8, 32]),
                    op=AL.mult)
                # self-loop contribution for this block
                hself = fp.tile([P, 256], bf16, tag="hself")
                nc.sync.dma_start(out=hself[:],
                                  in_=tshard[b * P:(b + 1) * P, 0:256])
                sl = sp.tile([P, 8], f32, tag="sl")
                nc.vector.tensor_tensor(out=sl[:],
                                        in0=sa_sb[:, b * 16:b * 16 + 8],
                                        in1=sa_sb[:, b * 16 + 8:b * 16 + 16],
                                        op=AL.add)
                sl2 = sp.tile([P, 8], f32, tag="sl2")
                nc.vector.tensor_scalar_mul(sl2[:], sl[:], 0.2)
                sl3 = sp.tile([P, 8], f32, tag="sl3")
                nc.vector.tensor_tensor(out=sl3[:], in0=sl[:], in1=sl2[:],
                                        op=AL.max)
                slt = mp.tile([P, 264], bf16, tag="slt")
                nc.scalar.activation(slt[:, 256:264], sl3[:], AF.Exp,
                                     bias=msh_sb[:])
                nc.vector.tensor_tensor(
                    out=slt[:, 0:256].rearrange("p (h w) -> p h w", h=8),
                    in0=hself[:].rearrange("p (h w) -> p h w", h=8),
                    in1=slt[:, 256:264].unsqueeze(-1)
                        .to_broadcast([P, 8, 32]),
                    op=AL.mult)
                # scatter-accumulate: self-loop first, then edge chunks
                pblk = pbp.tile([P, 264], f32, tag="pblk")
                nc.tensor.matmul(out=pblk[:], lhsT=identb_sb[:], rhs=slt[:],
                                 start=True, stop=False)
                for ch in range(NCH):
                    nc.tensor.matmul(out=pblk[:],
                                     lhsT=oh[:, ch * P:(ch + 1) * P],
                                     rhs=msg[:, ch * 264:(ch + 1) * 264],
                                     start=False, stop=(ch == NCH - 1))
                # block finalize: normalize, bias, elu
                den = sp.tile([P, 8], f32, tag="den")
                nc.vector.tensor_scalar_add(den[:], pblk[:, 256:264], 1e-16)
                rcp = sp.tile([P, 8], f32, tag="rcp")
                nc.vector.reciprocal(rcp[:], den[:])
                xb = fp.tile([P, 256], f32, tag="xb")
                nc.vector.tensor_tensor(
                    out=xb[:].rearrange("p (h c) -> p h c", h=8),
                    in0=pblk[:, 0:256].rearrange("p (h c) -> p h c", h=8),
                    in1=rcp[:].unsqueeze(-1).to_broadcast([P, 8, 32]),
                    op=AL.mult)
                nc.vector.tensor_tensor(out=xb[:], in0=xb[:], in1=b1_sb[:],
                                        op=AL.add)
                m0 = fp.tile([P, 256], f32, tag="m0")
                nc.vector.tensor_scalar(out=m0[:], in0=xb[:], scalar1=0.0,
                                        scalar2=None, op0=AL.min)
                ex = fp.tile([P, 256], f32, tag="ex")
                nc.scalar.activation(ex[:], m0[:], AF.Exp)
                nc.vector.tensor_scalar_add(ex[:], ex[:], -1.0)
                nc.vector.tensor_scalar(out=m0[:], in0=xb[:], scalar1=0.0,
                                        scalar2=None, op0=AL.max)
                h1b = fp.tile([P, 256], bf16, tag="h1b")
                nc.vector.tensor_tensor(out=h1b[:], in0=m0[:], in1=ex[:],
                                        op=AL.add)
                # transposes for layer-2 matmul
                for half, dstt in ((0, h1t0), (1, h1t1)):
                    pst2 = ptp.tile([P, P], bf16, tag="pst")
                    nc.tensor.transpose(
                        out=pst2[:],
                        in_=h1b[:, half * P:(half + 1) * P],
                        identity=identb_sb[:])
                    nc.scalar.activation(dstt[:, b * P:(b + 1) * P], pst2[:],
                                         AF.Copy)

            # -------------------------------------------- P4: layer-2 nodes
            for b in range(NB):
                p2 = pap.tile([P, 42], f32, tag="psc")
                nc.tensor.matmul(out=p2[:], lhsT=h1t0[:, b * P:(b + 1) * P],
                                 rhs=w2_sb[:, 0:42], start=True, stop=False)
                nc.tensor.matmul(out=p2[:], lhsT=h1t1[:, b * P:(b + 1) * P],
                                 rhs=w2_sb[:, 42:84], start=False, stop=True)
                h2x = sp.tile([P, 42], bf16, tag="h2x")
                nc.scalar.activation(h2x[:], p2[:], AF.Copy)
                nc.sync.dma_start(out=t2shard[b * P:(b + 1) * P, 0:42],
                                  in_=h2x[:])
                nc.vector.tensor_copy(sa2_sb[:, b * 2:(b + 1) * 2],
                                      h2x[:, 40:42])
                nc.vector.tensor_copy(h2own_sb[:, b * 40:(b + 1) * 40],
                                      h2x[:, 0:40])

            # -------------------------------------------- P5: allgather table2
            nc.gpsimd.collective_compute(
                "AllGather", AL.bypass, replica_groups=groups,
                ins=[t2shard[:]], outs=[t2full[:]])

            # -------------------------------------------- P6: L2 edge phase
            for b in range(NB):
                g2 = g2p.tile([P, NCH * TROW2], bf16, tag="g2")
                nc.gpsimd.dma_gather(
                    g2[:, 0:C1 * TROW2].rearrange("p (c w) -> p c w", c=C1),
                    t2full[0:HALF, :],
                    slo_sb[:, b * C1 * 8:(b + 1) * C1 * 8],
                    C1 * P, rn_lo, TROW2, single_packet=False, queue_num=0)
                nc.gpsimd.dma_gather(
                    g2[:, C1 * TROW2:].rearrange("p (c w) -> p c w", c=C2),
                    t2full[HALF:NCORES * SHP, :],
                    shi_sb[:, b * C2 * 8:(b + 1) * C2 * 8],
                    C2 * P, rn_hi, TROW2, single_packet=False, queue_num=1)
                ga2 = gap.tile([P, NCH * P], bf16, tag="ga")
                nc.gpsimd.dma_gather(
                    ga2[:, 0:C1 * P].rearrange("p (c w) -> p c w", c=C1),
                    t2shard[b * P:(b + 1) * P, :],
                    sw_sb[:, b * NCH * 8:b * NCH * 8 + C1 * 8],
                    C1 * P, rn_lo, TROW2, single_packet=False, queue_num=2)
                nc.gpsimd.dma_gather(
                    ga2[:, C1 * P:].rearrange("p (c w) -> p c w", c=C2),
                    t2shard[b * P:(b + 1) * P, :],
                    sw_sb[:, b * NCH * 8 + C1 * 8:(b + 1) * NCH * 8],
                    C2 * P, rn_hi, TROW2, single_packet=False, queue_num=3)
                g23 = g2[:].rearrange("p (c w) -> p c w", c=NCH)
                ga23 = ga2[:].rearrange("p (c w) -> p c w", c=NCH)
                oh = ohp.tile([P, NCH * P], bf16, tag="oh")
                nc.vector.tensor_tensor(
                    out=oh[:].rearrange("p (c s) -> p c s", c=NCH),
                    in0=ld_sb[:, b * NCH:(b + 1) * NCH].unsqueeze(-1)
                        .to_broadcast([P, NCH, P]),
                    in1=iota_sb[:].unsqueeze(1).to_broadcast([P, NCH, P]),
                    op=AL.is_equal)
                esc = sp.tile([P, NCH], f32, tag="esc1")
                nc.vector.tensor_tensor(
                    out=esc[:].unsqueeze(-1),
                    in0=g23[:, :, 40:41], in1=ga23[:, :, 41:42], op=AL.add)
                es2 = sp.tile([P, NCH], f32, tag="es21")
                nc.vector.tensor_scalar_mul(es2[:], esc[:], 0.2)
                es3 = sp.tile([P, NCH], f32, tag="es31")
                nc.vector.tensor_tensor(out=es3[:], in0=esc[:], in1=es2[:],
                                        op=AL.max)
                msg2 = mp.tile([P, NCH * 41], bf16, tag="msg2")
                m23 = msg2[:].rearrange("p (c w) -> p c w", c=NCH)
                nc.scalar.activation(m23[:, :, 40:41],
                                     es3[:].unsqueeze(-1),
                                     AF.Exp, bias=msh_sb[:])
                nc.vector.tensor_tensor(
                    out=m23[:, :, 0:40], in0=g23[:, :, 0:40],
                    in1=m23[:, :, 40:41].to_broadcast([P, NCH, 40]),
                    op=AL.mult)
                # self-loop contribution
                sl = sp.tile([P, 1], f32, tag="sl1")
                nc.vector.tensor_tensor(out=sl[:],
                                        in0=sa2_sb[:, b * 2:b * 2 + 1],
                                        in1=sa2_sb[:, b * 2 + 1:b * 2 + 2],
                                        op=AL.add)
                sl2 = sp.tile([P, 1], f32, tag="sl21")
                nc.vector.tensor_scalar_mul(sl2[:], sl[:], 0.2)
                sl3 = sp.tile([P, 1], f32, tag="sl31")
                nc.vector.tensor_tensor(out=sl3[:], in0=sl[:], in1=sl2[:],
                                        op=AL.max)
                slt2 = mp.tile([P, 41], bf16, tag="slt2")
                nc.scalar.activation(slt2[:, 40:41], sl3[:], AF.Exp,
                                     bias=msh_sb[:])
                nc.vector.tensor_tensor(
                    out=slt2[:, 0:40],
                    in0=h2own_sb[:, b * 40:(b + 1) * 40],
                    in1=slt2[:, 40:41].to_broadcast([P, 40]),
                    op=AL.mult)
                pblk2 = pbp.tile([P, 41], f32, tag="pblk")
                nc.tensor.matmul(out=pblk2[:], lhsT=identb_sb[:],
                                 rhs=slt2[:], start=True, stop=False)
                for ch in range(NCH):
                    nc.tensor.matmul(out=pblk2[:],
                                     lhsT=oh[:, ch * P:(ch + 1) * P],
                                     rhs=msg2[:, ch * 41:(ch + 1) * 41],
                                     start=False, stop=(ch == NCH - 1))
                den = sp.tile([P, 1], f32, tag="den1")
                nc.vector.tensor_scalar_add(den[:], pblk2[:, 40:41], 1e-16)
                rcp = sp.tile([P, 1], f32, tag="rcp1")
                nc.vector.reciprocal(rcp[:], den[:])
                o2 = o2_sb[:, b * NCLASS:(b + 1) * NCLASS]
                nc.vector.tensor_tensor(out=o2, in0=pblk2[:, 0:40],
                                        in1=rcp[:].to_broadcast([P, 40]),
                                        op=AL.mult)
                nc.vector.tensor_tensor(out=o2, in0=o2, in1=b2_sb[:], op=AL.add)
                nc.vector.tensor_reduce(out=mmax_sb[:, b:b + 1], in_=o2,
                                        axis=mybir.AxisListType.X, op=AL.max)
                negm = sp.tile([P, 1], f32, tag="negm")
                nc.vector.tensor_scalar_mul(negm[:], mmax_sb[:, b:b + 1], -1.0)
                et = sp.tile([P, NCLASS], f32, tag="et")
                nc.scalar.activation(et[:], o2, AF.Exp, bias=negm[:],
                                     accum_out=ssum_sb[:, b:b + 1])

            # batched log + final outputs
            lnS = perm.tile([P, NB], f32)
            nc.scalar.activation(lnS[:], ssum_sb[:], AF.Ln)
            q = perm.tile([P, NB], f32)
            nc.vector.tensor_tensor(out=q[:], in0=mmax_sb[:], in1=lnS[:],
                                    op=AL.add)
            for b in range(NB):
                o2 = o2_sb[:, b * NCLASS:(b + 1) * NCLASS]
                lsb = sp.tile([P, NCLASS], f32, tag="lsb")
                nc.vector.tensor_tensor(out=lsb[:], in0=o2,
                                        in1=q[:, b:b + 1].to_broadcast([P, 40]),
                                        op=AL.subtract)
                nc.sync.dma_start(out=fin[b * P:(b + 1) * P, :], in_=o2)
                nc.sync.dma_start(out=lsm[b * P:(b + 1) * P, :], in_=lsb[:])

    # dma_gather needs the 'mlp' GPSIMD ucode library; raw Bass doesn't run
    # Bacc's insert_library_loads pass, so run it (plus ISA codegen) here.
    from concourse.library_config import all_libraries, standard
    import bass_rust as _br
    mask = {}
    for lib_ in all_libraries:
        for it in lib_.instructions:
            mask[it] = mask.get(it, 0) | (1 << lib_.index)
    _br.insert_library_loads(nc, mask, len(all_libraries), standard.index)
    mybir.codegen_inst_isa_subclasses(nc)
    return nc


# ---------------------------------------------------------------------------
def _host_prep(x, edge_index, W1, att_src1, att_dst1, b1, W2, att_src2,
               att_dst2, b2):
    x = np.asarray(x, dtype=np.float32)
    ei = np.asarray(edge_index)
    W1 = np.asarray(W1, np.float32)
    W2 = np.asarray(W2, np.float32)
    a_s1 = np.asarray(att_src1, np.float32)
    a_d1 = np.asarray(att_dst1, np.float32)
    a_s2 = np.asarray(att_src2, np.float32).reshape(-1)
    a_d2 = np.asarray(att_dst2, np.float32).reshape(-1)
    b1 = np.asarray(b1, np.float32)
    b2 = np.asarray(b2, np.float32)

    import ml_dtypes
    bf = ml_dtypes.bfloat16

    # self-loops are handled analytically on-device; edge stream = real edges
    src = np.asarray(ei[0], np.int64)
    dst = np.asarray(ei[1], np.int64)

    core = dst // SH
    local = dst - core * SH
    block = local // P
    slot = (local % P).astype(np.int64)
    cb = core * NB + block
    remap = ((src // SH) * SHP + (src % SH)).astype(np.int64)
    hi = (remap >= HALF).astype(np.int64)

    grp = cb * 2 + hi
    order = np.argsort(grp, kind="stable")
    grp_s = grp[order]
    remap_s = remap[order]
    slot_s = slot[order]
    cb_s = cb[order]
    hi_s = hi[order]

    gcounts = np.bincount(grp_s, minlength=NCORES * NB * 2)
    goffs = np.zeros(NCORES * NB * 2 + 1, np.int64)
    goffs[1:] = np.cumsum(gcounts)
    pos = np.arange(order.size, dtype=np.int64) - goffs[grp_s]

    cntlo = gcounts[0::2].reshape(NCORES, NB)
    cnthi = gcounts[1::2].reshape(NCORES, NB)
    C1 = int(np.ceil(cntlo.max() / P))
    C2 = int(np.ceil(cnthi.max() / P))
    NCH = C1 + C2
    TOT = NB * NCH

    corev = cb_s // NB
    blockv = cb_s % NB
    k = pos + hi_s * (C1 * P)          # global slot index within block
    lane = k % P
    chunk = k // P

    ld_all = np.full((NCORES, P, TOT), -1.0, np.float32)
    ld_all[corev, lane, blockv * NCH + chunk] = slot_s.astype(np.float32)

    # int16 index arrays, wrapped 16 ways: flat[j] -> arr[j%16, j//16]
    slo = np.zeros((NCORES, P, NB * C1 * 8), np.int16)
    shi_a = np.zeros((NCORES, P, NB * C2 * 8), np.int16)
    sw = np.zeros((NCORES, P, NB * NCH * 8), np.int16)

    # indices live in partitions 0-15 (RX Q7 core) AND 16-31 (TX Q7 core)
    lo_m = hi_s == 0
    hi_m = hi_s == 1
    for poff in range(0, P, 16):
        slo[corev[lo_m], poff + pos[lo_m] % 16,
            blockv[lo_m] * (C1 * 8) + pos[lo_m] // 16] = remap_s[lo_m]
        shi_a[corev[hi_m], poff + pos[hi_m] % 16,
              blockv[hi_m] * (C2 * 8) + pos[hi_m] // 16] = \
            (remap_s[hi_m] - HALF)
        sw[corev, poff + k % 16, blockv * (NCH * 8) + k // 16] = slot_s

    # weights
    wa_s1 = np.zeros((NFEAT, 8), np.float32)
    wa_d1 = np.zeros((NFEAT, 8), np.float32)
    for h in range(NHEAD):
        wa_s1[:, h] = W1[:, h * HID:(h + 1) * HID] @ a_s1[h]
        wa_d1[:, h] = W1[:, h * HID:(h + 1) * HID] @ a_d1[h]
    wcat1 = np.concatenate([W1, wa_s1, wa_d1], axis=1)          # [256, 272]
    w2cat = np.concatenate([W2, (W2 @ a_s2)[:, None],
                            (W2 @ a_d2)[:, None]], axis=1)      # [256, 42]

    iota_row = np.broadcast_to(np.arange(P, dtype=np.float32),
                               (P, P)).astype(bf)
    ident = np.eye(P, dtype=bf)
    b1rep = np.broadcast_to(b1, (P, 256)).copy()
    b2rep = np.broadcast_to(b2, (P, NCLASS)).copy()

    in_maps = []
    for c in range(NCORES):
        xs = np.zeros((SHP, NFEAT), np.float32)
        xs[:SH] = x[c * SH:(c + 1) * SH]
        in_maps.append({
            "xT": np.ascontiguousarray(xs.T).astype(bf),
            "wcat1": wcat1.astype(bf), "w2cat": w2cat.astype(bf),
            "b1rep": b1rep, "b2rep": b2rep,
            "iota_row": iota_row, "ident": ident,
            "ld": np.ascontiguousarray(ld_all[c]).astype(bf),
            "slo": np.ascontiguousarray(slo[c]),
            "shi": np.ascontiguousarray(shi_a[c]),
            "sw": np.ascontiguousarray(sw[c]),
        })
    return (C1, C2), in_maps


def kernel(**inputs):
    _install_bir_patch()
    from concourse.bass_utils import run_bass_kernel_spmd

    key, in_maps = _host_prep(
        inputs["x"], inputs["edge_index"], inputs["W1"], inputs["att_src1"],
        inputs["att_dst1"], inputs["b1"], inputs["W2"], inputs["att_src2"],
        inputs["att_dst2"], inputs["b2"])

    if key not in _prog_cache:
        _prog_cache[key] = _build_program(key)
    nc = _prog_cache[key]

    res = run_bass_kernel_spmd(nc, in_maps, list(range(NCORES)))
    fin = np.concatenate([res.results[c]["fin"][:SH] for c in range(NCORES)])
    lsm = np.concatenate([res.results[c]["lsm"][:SH] for c in range(NCORES)])
    return fin, lsm
